# revision 39
# baseline (speedup 1.0000x reference)
"""Multi-head self-attention on 8 Trainium2 NeuronCores.

Problem: x(2,2048,1024), 16 heads of 64, fp32 reference. Sharding: batch (2) x
head-groups (4 groups of 4 heads). Each core computes Q/K/V projections for its
256 head-dims, attention for its 4 heads, and a partial out-projection (its 256
rows of Wo). Host sums the 4 group-partials per batch (the tensor-parallel
all-reduce) and adds bo.

Kernel layout (per core), v3:
  All matmul operands fp16 (PSUM fp32; softmax stats fp32).
  xT [1024,2048]; QT/KT [256,2048] head-pair per 128-partition tile;
  V natural [2048,256] with a ones column per head (V_aug [.,65]).

  Emission order targets an early softmax start and zero PE idle:
    1. K proj m0 (k-outer, DMA-paced), then Q proj m0 -> first scores ~12us.
    2. 8 ticks (pair-major, 512-wide s-chunks): per st-tile emit
       scoresT[t,s] (two K=64 row-group matmuls sharing a psum tile), exp on
       ScalarE into fp16 `at` tiles, and the PREVIOUS tick's attnV.
       m1 Q/K proj and V proj chunks are interleaved into the early ticks'
       st slots as background PE work.
    3. attnV v3: at-tile stationary [128t,128s], V_aug moving [128t,65] ->
       psum ctx[s_sub 128, 65] accumulated over the 16 t-chunks; col 64 is
       the softmax denominator (65-col streams vs 512: 2x fewer PE cycles).
    4. Normalize = DVE reciprocal of the denom column + per-partition-scalar
       multiply on GpSimd (no transpose/broadcast dance), then PE transposes
       ctx[s,dh] -> ctxT[dh,s] via identity matmul; GpSimd evacuates to ctxN.
    5. Out projection per s-chunk once both pairs' ctxN landed; result is
       DMA'd to DRAM directly from PSUM (no evacuation op).
  Input DMAs are split across the SP/Pool queues (x) and SP (weights) so the
  first matmul starts ~2.5us in; out DMAs ride the Pool queue (25ns issue).
"""

import sys

sys.path.insert(0, "/opt/trn_rl_repo")

import numpy as np

import concourse.bacc as bacc
import concourse.mybir as mybir
import concourse.tile as tile
from concourse import bass_utils

N_CORES = 8
B, S, D = 2, 2048, 1024
H_LOC = 4          # heads per core
DH = 64            # head dim
DG = H_LOC * DH    # 256 group dims per core
KC = D // 128      # 8 contraction chunks over D
ST = S // 128      # 16 s/t tiles
SC = S // 512      # 4 512-wide s chunks
MT = DG // 128     # 2 m-tiles of group dims

F32 = mybir.dt.float32
F16 = mybir.dt.float16


def _build_program(reps=1, num_devices=N_CORES, phases=("proj", "attn", "out"),
                   debug_ctx=False):
    nc = bacc.Bacc("TRN2", target_bir_lowering=False, debug=False,
                   num_devices=num_devices)
    dbg_d = (nc.dram_tensor("dbg", [128, MT, S], F16, kind="ExternalOutput")
             if debug_ctx else None)

    xT_d = nc.dram_tensor("xT", [KC, 2, 128, S // 2], F16, kind="ExternalInput")
    wq_d = nc.dram_tensor("wq", [KC, 128, DG], F16, kind="ExternalInput")
    wk_d = nc.dram_tensor("wk", [KC, 128, DG], F16, kind="ExternalInput")
    wv_d = nc.dram_tensor("wv", [KC, 128, DG], F16, kind="ExternalInput")
    bq_d = nc.dram_tensor("bq", [MT, 128, 1], F32, kind="ExternalInput")
    bk_d = nc.dram_tensor("bk", [MT, 128, 1], F32, kind="ExternalInput")
    bv_d = nc.dram_tensor("bv", [1, DG], F32, kind="ExternalInput")
    wo_d = nc.dram_tensor("wo", [MT, 128, D], F16, kind="ExternalInput")
    eye_d = nc.dram_tensor("eye", [128, 128], F32, kind="ExternalInput")
    out_d = nc.dram_tensor("out", [S, D], F16, kind="ExternalOutput")

    with tile.TileContext(nc) as tc:
      for _rep in range(reps):
        with (
            tc.tile_pool(name="wpool", bufs=1) as wpool,
            tc.tile_pool(name="mpool", bufs=1) as mpool,
            tc.tile_pool(name="apool", bufs=1) as apool,
            tc.tile_pool(name="psum", bufs=3, space="PSUM") as pp,
        ):
            # ---- weights / biases / x ----
            wq_t = wpool.tile([128, KC, DG], F16)
            wk_t = wpool.tile([128, KC, DG], F16)
            wv_t = wpool.tile([128, KC, DG], F16)
            wo_t = wpool.tile([128, MT, D], F16)
            bq_t = wpool.tile([128, MT], F32)
            bk_t = wpool.tile([128, MT], F32)
            bv_row = wpool.tile([1, DG], F32)
            bv_bc = wpool.tile([128, H_LOC, DH], F32)
            eye_t = wpool.tile([128, 128], F32)
            xT_t = mpool.tile([128, KC, S], F16)

            # ---- persistent intermediates ----
            qT_t = mpool.tile([128, MT, S], F16)    # [dg_row, mt, s]
            kT_t = mpool.tile([128, MT, S], F16)
            vaug = mpool.tile([128, ST, H_LOC, DH + 1], F16)
            ctxN = mpool.tile([128, MT, S], F16)    # normalized ctx^T

            # DMA queues: x split across SP + Pool SWDGE (two queues keep
            # the chunk cadence ahead of the PE); all weights on the Act
            # HWDGE (wk/wq interleaved first so m0 keeps up); small tensors
            # on Pool behind x.
            for k in range(KC):
                nc.sync.dma_start(xT_t[:, k, 0 : S // 2], xT_d.ap()[k, 0])
                nc.gpsimd.dma_start(xT_t[:, k, S // 2 : S], xT_d.ap()[k, 1])
                nc.scalar.dma_start(wk_t[:, k, :], wk_d.ap()[k])
                nc.scalar.dma_start(wq_t[:, k, :], wq_d.ap()[k])
            for k in range(KC):
                nc.scalar.dma_start(wv_t[:, k, :], wv_d.ap()[k])
            nc.gpsimd.dma_start(bk_t[:, 0:1], bk_d.ap()[0])
            nc.gpsimd.dma_start(bk_t[:, 1:2], bk_d.ap()[1])
            nc.gpsimd.dma_start(bq_t[:, 0:1], bq_d.ap()[0])
            nc.gpsimd.dma_start(bq_t[:, 1:2], bq_d.ap()[1])
            nc.gpsimd.dma_start(bv_row[:], bv_d.ap())
            nc.gpsimd.memset(vaug[:], 1.0)
            for h in range(H_LOC):
                nc.gpsimd.partition_broadcast(
                    bv_bc[:, h, :], bv_row[:, h * DH : h * DH + DH])
            nc.gpsimd.dma_start(eye_t[:], eye_d.ap())
            for m in range(MT):
                nc.gpsimd.dma_start(wo_t[:, m, :], wo_d.ap()[m])

            # ---- m0 projections: K and Q interleaved per x-chunk so the
            # PE consumes each chunk as it lands (K psums ride the idle ctx
            # ring, Q psums the ps ring -> 4 live [128,1024] accumulators) ----
            def qk_psums(m, tag):
                return [
                    pp.tile([128, 1024], F32, tag=tag, name=f"ps_qk{m}{h}")
                    for h in range(2)
                ]

            def qk_mm(ps, w_t, m, half, k):
                for i in range(2):
                    sc = 2 * half + i
                    nc.tensor.matmul(
                        ps[:, i * 512 : i * 512 + 512],
                        w_t[:, k, m * 128 : m * 128 + 128],
                        xT_t[:, k, sc * 512 : sc * 512 + 512],
                        start=(k == 0), stop=(k == KC - 1),
                    )

            def qk_evac(ps, o_t, b_t, m, half, eng="v"):
                for i in range(2):
                    sc = 2 * half + i
                    if eng == "a":
                        # ScalarE is idle pre-softmax; Copy with bias add
                        nc.scalar.activation(
                            o_t[:, m, sc * 512 : sc * 512 + 512],
                            ps[:, i * 512 : i * 512 + 512],
                            mybir.ActivationFunctionType.Identity,
                            bias=b_t[:, m : m + 1])
                    else:
                        nc.vector.tensor_scalar_add(
                            o_t[:, m, sc * 512 : sc * 512 + 512],
                            ps[:, i * 512 : i * 512 + 512],
                            b_t[:, m : m + 1])

            # K m0 first (scores need the full kT); its DVE evacuation
            # overlaps the Q-half0 matmuls, so the first scores launch right
            # after Q-half0's evac with zero PE idle. Q-half1 is deferred
            # into tick 0 as background work.
            ps_k0 = [
                pp.tile([128, 1024], F32, tag="ctx", bufs=1, name="ps_k00"),
                pp.tile([128, 1024], F32, tag="ps", name="ps_k01"),
            ]
            for k in range(KC):
                for h in range(2):
                    qk_mm(ps_k0[h], wk_t, 0, h, k)
            for h in range(2):
                qk_evac(ps_k0[h], kT_t, bk_t, 0, h, eng="v")
            ps_q00 = pp.tile([128, 1024], F32, tag="ps", name="ps_q00")
            for k in range(KC):
                qk_mm(ps_q00, wq_t, 0, 0, k)
            qk_evac(ps_q00, qT_t, bq_t, 0, 0, eng="v")

            # ---- background PE work scheduled into tick st-slots ----
            def emit_v_chunk(st):
                ps_v = pp.tile([128, H_LOC, DH], F32, tag="ps",
                               name=f"ps_v{st}")
                for k in range(KC):
                    nc.tensor.matmul(
                        ps_v[:],
                        xT_t[:, k, st * 128 : st * 128 + 128],
                        wv_t[:, k, :],
                        start=(k == 0), stop=(k == KC - 1),
                    )
                nc.vector.tensor_add(
                    vaug[:, st, :, 0:DH], ps_v[:], bv_bc[:])

            def emit_qk_quarter(which, m, sc):
                # one 512-wide s-chunk of a Q or K projection m-tile
                w_t, o_t, b_t = ((wk_t, kT_t, bk_t) if which == "k"
                                 else (wq_t, qT_t, bq_t))
                ps = pp.tile([128, 512], F32, tag="ps",
                             name=f"ps_{which}{m}{sc}")
                for k in range(KC):
                    nc.tensor.matmul(
                        ps[:],
                        w_t[:, k, m * 128 : m * 128 + 128],
                        xT_t[:, k, sc * 512 : sc * 512 + 512],
                        start=(k == 0), stop=(k == KC - 1),
                    )
                nc.vector.tensor_scalar_add(
                    o_t[:, m, sc * 512 : sc * 512 + 512], ps[:],
                    b_t[:, m : m + 1])

            slot_work = {}

            def add_work(t, st, fn):
                slot_work.setdefault((t, st), []).append(fn)

            # all 16 V chunks inside tick 0: the j-pass attnV of tick 0
            # consumes every vaug chunk in tick 1's first slots
            for i in range(ST):
                add_work(0, i, lambda st=i: emit_v_chunk(st))
            add_work(1, 3, lambda: emit_qk_quarter("k", 1, 0))
            add_work(1, 7, lambda: emit_qk_quarter("k", 1, 1))
            add_work(1, 11, lambda: emit_qk_quarter("q", 0, 2))
            add_work(2, 3, lambda: emit_qk_quarter("k", 1, 2))
            add_work(2, 7, lambda: emit_qk_quarter("k", 1, 3))
            add_work(2, 11, lambda: emit_qk_quarter("q", 0, 3))
            add_work(3, 3, lambda: emit_qk_quarter("q", 1, 0))
            add_work(4, 7, lambda: emit_qk_quarter("q", 1, 1))
            add_work(5, 7, lambda: emit_qk_quarter("q", 1, 2))
            add_work(6, 7, lambda: emit_qk_quarter("q", 1, 3))

            # ---- deferred per-tick epilogue (normalize/transpose/outproj),
            # spread into a later tick's st slots so scores never stall ----
            def emit_recip(vctx, rec):
                # per-group contiguous reads: a strided [128,4,1] psum read
                # mis-lowers (all groups got the last j's denominator)
                for i in range(2):
                    for j in range(4):
                        g = i * 4 + j
                        nc.vector.reciprocal(
                            rec[:, g : g + 1], vctx[:, i, j, 64:65])

            def emit_norm_j(vctx, rec, ctx_s, j):
                for i in range(2):
                    g = i * 4 + j
                    nc.vector.tensor_scalar_mul(
                        ctx_s[:, g * 64 : g * 64 + 64],
                        vctx[:, i, j, 0:64],
                        rec[:, g : g + 1])

            def emit_transp_j(ctx_s, tps, j):
                # PSUM APs must start at partition 0: both heads' transposes
                # land side by side in one [64, 1024] psum tile, head-major,
                # so each head evacuates as one contiguous [64, 512] copy
                for i in range(2):
                    g = i * 4 + j
                    nc.tensor.transpose(
                        tps[:, i * 512 + j * 128 : i * 512 + j * 128 + 128],
                        ctx_s[:, g * 64 : g * 64 + 64],
                        eye_t[:])

            def emit_outproj(st_o, qi, tail=False):
                ps_o = pp.tile([128, 1024], F32, tag="ps", name="ps_o")
                for m in range(MT):
                    for n in range(2):
                        nsl = slice(n * 512, n * 512 + 512)
                        nc.tensor.matmul(
                            ps_o[:, nsl],
                            ctxN[:, m, st_o * 128 : st_o * 128 + 128],
                            wo_t[:, m, nsl],
                            start=(m == 0), stop=(m == MT - 1),
                        )
                o_t = apool.tile([128, 1024], F16, tag="ot", bufs=4,
                                 name="o_t")
                if tail:
                    # softmax is drained by now: ScalarE and its HWDGE are
                    # free to help the epilogue chain
                    nc.vector.tensor_copy(o_t[:, 0:512], ps_o[:, 0:512])
                    nc.scalar.activation(
                        o_t[:, 512:1024], ps_o[:, 512:1024],
                        mybir.ActivationFunctionType.Copy)
                    eng = nc.scalar if qi % 2 == 0 else nc.sync
                else:
                    nc.vector.tensor_copy(o_t[:], ps_o[:])
                    eng = nc.sync
                eng.dma_start(
                    out_d.ap()[st_o * 128 : st_o * 128 + 128, :], o_t[:])

            def emit_epilogue(vpair, vsc, vctx, t_next, inline_out=False):
                # normalize + transpose now (frees the ctx psum quickly);
                # out-projections are either deferred into the next tick's
                # st slots or emitted inline (tail)
                rec = apool.tile([128, 8], F32, tag="rec", bufs=3, name="rec")
                ctx_s = apool.tile([128, 512], F32, tag="ctxs", bufs=3,
                                   name="ctx_s")
                tps = pp.tile([64, 1024], F32, tag="ps", name="tps")
                in_tail = inline_out or t_next >= len(ticks)
                emit_recip(vctx, rec)
                for j in range(4):
                    emit_norm_j(vctx, rec, ctx_s, j)
                    emit_transp_j(ctx_s, tps, j)
                for i in range(2):
                    nc.vector.tensor_copy(
                        ctxN[i * 64 : i * 64 + 64, vpair,
                             vsc * 512 : vsc * 512 + 512],
                        tps[:, i * 512 : i * 512 + 512])
                for j in range(4):
                    if vpair == 1 and "out" in phases:
                        if inline_out:
                            emit_outproj(vsc * 4 + j, j, tail=True)
                        else:
                            add_work(t_next, 1 + 4 * j,
                                     lambda j=j, tl=in_tail: emit_outproj(
                                         vsc * 4 + j, j, tail=tl))

            # ---- attention ticks ----
            ticks = [(p, sc) for p in range(2) for sc in range(SC)]
            prev = None  # (pair, sc, at_tiles)

            def attn_v(vpair, tiles, st, dst):
                # one accumulation group per psum bank at a time: sweep the
                # full contraction for subtile j (= st//4) before starting
                # j+1's groups, so a start=True never zeroes a pending
                # sibling group in the same 2KB zero region
                j = st // 4
                for k in range(4 * (st % 4), 4 * (st % 4) + 4):
                    for i in range(2):
                        nc.tensor.matmul(
                            dst[:, i, j, :],
                            tiles[k][:, i * 512 + j * 128
                                     : i * 512 + j * 128 + 128],
                            vaug[:, k, 2 * vpair + i, :],
                            start=(k == 0), stop=(k == ST - 1),
                        )

            tail_out = []  # outproj closures that spill past the last tick
            for t in range(len(ticks)):
                pair, sc = ticks[t]
                vctx = None
                if prev is not None:
                    vctx = pp.tile([128, 2, 4, 65], F32,
                                   padded_shape=[128, 2, 4, 128],
                                   tag="ctx", bufs=1, name="ctx_ps")
                cur = []
                for st in range(ST):
                    tsl = slice(st * 128, st * 128 + 128)
                    ssl = slice(sc * 512, sc * 512 + 512)
                    ps_s = pp.tile([128, 1024], F32, tag="ps", name="ps_s")
                    # two K=64 matmuls in disjoint PE row groups
                    nc.tensor.matmul(
                        ps_s[:, 0:512],
                        kT_t[0:64, pair, tsl], qT_t[0:64, pair, ssl])
                    nc.tensor.matmul(
                        ps_s[:, 512:1024],
                        kT_t[64:128, pair, tsl], qT_t[64:128, pair, ssl])
                    at = apool.tile([128, 1024], F16, tag="attnT",
                                    bufs=20, name="at")
                    nc.scalar.activation(
                        at[:], ps_s[:],
                        mybir.ActivationFunctionType.Exp, scale=0.125)
                    cur.append(at)
                    if prev is not None:
                        attn_v(prev[0], prev[2], st, vctx)
                    for fn in slot_work.get((t, st), ()):
                        fn()
                if prev is not None:
                    emit_epilogue(prev[0], prev[1], vctx, t + 1)
                prev = (pair, sc, cur)
            tail_out = [
                fn for st in range(ST)
                for fn in slot_work.get((len(ticks), st), ())
            ]

            # ---- tail: last tick's attnV chases the draining exps, with
            # the previous epilogue's out-projections interleaved ----
            vctx = pp.tile([128, 2, 4, 65], F32,
                           padded_shape=[128, 2, 4, 128],
                           tag="ctx", bufs=1, name="ctx_ps_f")
            for k in range(ST):
                attn_v(prev[0], prev[2], k, vctx)
                if k % 4 == 3 and tail_out:
                    tail_out.pop(0)()
            for fn in tail_out:
                fn()
            emit_epilogue(prev[0], prev[1], vctx, None, inline_out=True)
            if dbg_d is not None:
                nc.sync.dma_start(dbg_d.ap(), ctxN[:])

    nc.compile()
    return nc


_CACHE = {}


def _get_program():
    if "nc" not in _CACHE:
        _CACHE["nc"] = _build_program()
    return _CACHE["nc"]


def _shard_inputs(x, Wq, bq, Wk, bk, Wv, bv, Wo):
    xT16 = [
        np.ascontiguousarray(x[b].T).astype(np.float16)
        .reshape(KC, 128, 2, S // 2).transpose(0, 2, 1, 3)
        .copy()
        for b in range(B)
    ]
    eye = np.eye(128, dtype=np.float32)
    in_maps = []
    for c in range(N_CORES):
        b, g = c // 4, c % 4
        gs = slice(g * DG, g * DG + DG)
        in_maps.append({
            "xT": xT16[b],
            "wq": np.ascontiguousarray(Wq[:, gs]).astype(np.float16).reshape(KC, 128, DG),
            "wk": np.ascontiguousarray(Wk[:, gs]).astype(np.float16).reshape(KC, 128, DG),
            "wv": np.ascontiguousarray(Wv[:, gs]).astype(np.float16).reshape(KC, 128, DG),
            "bq": np.ascontiguousarray(bq[gs]).astype(np.float32).reshape(MT, 128, 1),
            "bk": np.ascontiguousarray(bk[gs]).astype(np.float32).reshape(MT, 128, 1),
            "bv": np.ascontiguousarray(bv[gs]).astype(np.float32).reshape(1, DG),
            "wo": np.ascontiguousarray(Wo[gs, :]).astype(np.float16).reshape(MT, 128, D),
            "eye": eye,
        })
    return in_maps


def kernel(x, Wq, bq, Wk, bk, Wv, bv, Wo, bo, _trace=False, _trace_kwargs=None):
    x = np.asarray(x, dtype=np.float32)
    Wq, bq = np.asarray(Wq, np.float32), np.asarray(bq, np.float32)
    Wk, bk = np.asarray(Wk, np.float32), np.asarray(bk, np.float32)
    Wv, bv = np.asarray(Wv, np.float32), np.asarray(bv, np.float32)
    Wo, bo = np.asarray(Wo, np.float32), np.asarray(bo, np.float32)

    nc = _get_program()
    in_maps = _shard_inputs(x, Wq, bq, Wk, bk, Wv, bv, Wo)
    kwargs = {}
    if _trace:
        kwargs["trace"] = True
        kwargs.update(_trace_kwargs or {})
    res = bass_utils.run_bass_kernel_spmd(
        nc, in_maps, core_ids=list(range(N_CORES)), **kwargs)

    out = np.zeros((B, S, D), dtype=np.float32)
    for c in range(N_CORES):
        out[c // 4] += res.results[c]["out"].astype(np.float32)
    out += bo
    if _trace:
        kernel.last_result = res
    return out
